# revision 27
# baseline (speedup 1.0000x reference)
"""Trainium2 Bass kernel for sorted-segment sum+mean (segment_reduce).

out[g] = concat(mean_g, sum_g) over rows of nbr_fea grouped by sorted
segment_ids; out shape [num_segments, 2*D].

Strategy
--------
Rows are sorted by segment id, so each segment is a contiguous row range.
Segments are packed greedily into "chunks" of at most S=24 consecutive
segments and at most T*128 rows (T chosen to minimize total padded rows);
each chunk's rows are packed (on host) into T row-tiles of 128 rows, laid
out DMA-optimally as [batch][partition][chunk][tile][feat] so batches of
DMA_BATCH=6 chunks load as one fully contiguous ~17 KiB-per-partition DMA.
The two HW DGE rings (driven by the sync and scalar engines -- the only
engines that can drive them) alternate batches; at ~330-420 GB/s/core all
8 cores together sit at the chip-level HBM roofline, so total bytes is the
binding constraint.

The f32 features are rounded to bf16 on host (half the HBM bytes of f32;
measured output error ~1.6e-3 scale-relative vs the 2e-2 gate), so the
TensorEngine runs single-pass bf16 matmuls with x on the fast moving port
(~2 cols/cycle; never put the bulk operand on the ~1 col/cycle LDWEIGHTS
port).

On device, per 128-row tile, a one-hot matrix U[row, slot] = (rel_id ==
slot) is built on the VectorEngine (is_equal of a per-batch rel tile
broadcast against a tiny broadcast iota, one op per 6-chunk batch) in bf16
and used as the matmul *stationary* operand; the moving operand is the
[128 rows, 64] x tile:  psum[slot, :] += U.T @ x  accumulated per chunk in
PSUM fp32, with EPB=8 chunks packed side-by-side in one 2 KiB PSUM bank
(PSUM free-dim offsets are arbitrary; partition offsets are quadrant-
locked).  Per-batch rel tiles ride the same ring just before their x batch
(separate tiles because tile deps are whole-tile, not sub-range).

The epilogue is batched per 8-chunk bank and emitted one batch late so the
DMA-issuing engines never stall behind fresh compute: ACT copies the sums
out of PSUM through a strided rearrange view, DVE multiplies by the
host-baked broadcast 1/count row for the means, and staged results flush
via the SWDGE queue (last flush on a HW ring to shorten the tail).
Padding rows carry rel_id = -1 so their one-hot row is all zero; unused
slots produce zeros the host discards.

The kernel is compiled AFTER seeing the inputs, so the (data-dependent)
chunk plan is a compile-time constant; one SPMD program runs on all 8 cores.
"""

import ml_dtypes
import numpy as np

import concourse.bass as bass
import concourse.mybir as mybir
import concourse.tile as tile
from concourse import bass_utils

N_TOTAL = 4_194_304
D = 64                       # feature dim
G = 32_768                   # num segments
N_CORES = 8
S = 24                       # segment slots per chunk (psum partitions)
P = 128                      # rows per tile == SBUF partitions
DMA_BATCH = 6                # chunks per x dma_start (~17 KiB lines)
EPB = 8                      # chunks packed per PSUM bank (2 KiB = 8*64 f32)

F32 = mybir.dt.float32
BF16 = mybir.dt.bfloat16
I8 = mybir.dt.int8
NP_BF16 = ml_dtypes.bfloat16


def _split_syncs(nc, max_waits=1):
    """This container's walrus accepts at most one sync-wait per instruction;
    split extra waits onto preceding same-engine NoOps (engine stalls at each
    wait in turn, so semantics are identical)."""
    n_split = 0
    for f in nc.m.functions:
        for bb in f.blocks:
            new_insts = []
            for ins in bb.instructions:
                si = getattr(ins, "sync_info", None)
                waits = list(si.on_wait) if si is not None and si.on_wait else []
                if len(waits) > max_waits:
                    n_split += 1
                    extra = waits[:-max_waits]
                    for i in range(0, len(extra), max_waits):
                        nop = mybir.InstNoOp(
                            name=f"{ins.name}_wsplit{i}", ins=[], outs=[]
                        )
                        nop.engine = ins.engine
                        nop.sync_info = mybir.SyncInfo(
                            on_wait=extra[i : i + max_waits], on_update=[]
                        )
                        new_insts.append(nop)
                    si.on_wait = waits[-max_waits:]
                new_insts.append(ins)
            bb.instructions = new_insts
    return n_split


FLUSH_ALIGN = 48  # C must be a multiple of lcm(DMA_BATCH, EPB, FLUSH)


def _bsizes(C):
    """Ragged batch schedule: tiny head batches so the first matmuls start
    after ~1 chunk instead of 6 (tile deps are whole-tile), small tail
    batches so the last matmuls trail the final bytes closely."""
    assert (C - 12) % DMA_BATCH == 0
    return [1, 1, 2, 2] + [DMA_BATCH] * ((C - 12) // DMA_BATCH) + [4, 2]


def _build_bass(T, C, split_syncs=True):
    """Build the SPMD program: C chunks per core, T row-tiles per chunk."""
    nc = bass.Bass("TRN2", debug=False, num_devices=1)

    assert C % FLUSH_ALIGN == 0
    BSIZES = _bsizes(C)
    NB = len(BSIZES)
    # x stored as one dense [P, k*T*D] block per batch, concatenated: every
    # batch DMA is a single fully contiguous 2D block (a flat [P, C*T*D]
    # layout would stride each line ~540 KB apart and lose ~9% DMA rate)
    x_d = nc.dram_tensor("x", [P * C * T * D], BF16, kind="ExternalInput")
    rel_d = nc.dram_tensor("rel", [P, C * T], I8, kind="ExternalInput")
    iota_d = nc.dram_tensor("iota", [P, S], I8, kind="ExternalInput")
    recip_d = nc.dram_tensor("recip", [S, C], F32, kind="ExternalInput")
    out_d = nc.dram_tensor("out", [S, C * 2 * D], F32, kind="ExternalOutput")

    FLUSH = 2 * EPB  # chunks per output flush (= 2 psum banks)

    with tile.TileContext(nc) as tc:
        with (
            tc.tile_pool(name="const", bufs=1) as const_pool,
            tc.tile_pool(name="xin", bufs=8) as x_pool,
            tc.tile_pool(name="rel", bufs=NB) as rel_pool,
            tc.tile_pool(name="oh", bufs=3) as oh_pool,
            tc.tile_pool(name="outs", bufs=2) as out_pool,
            tc.tile_pool(name="ps", bufs=6, space="PSUM") as ps_pool,
        ):
            # constants: the tiny rel head + iota go FIRST on the two HW
            # rings (they gate the first one-hot, so SWDGE's ~70 GB/s would
            # stall the ramp); recip + the rel remainder trickle on SWDGE,
            # always ahead of use.  x then streams on both HW rings.
            iota_sb = const_pool.tile([P, S], I8)
            nc.scalar.dma_start(iota_sb[:], iota_d[:])
            recip_sb = const_pool.tile([S, C], F32)
            nc.gpsimd.dma_start(recip_sb[:], recip_d[:])

            pending = []  # (first chunk, ps bank) epilogues delayed a batch
            out_sb = None
            out_view = None
            flushed = 0

            def emit_epilogue(upto_excl):
                # one batched copy + one batched scale per 8-chunk PSUM bank
                nonlocal out_sb, out_view, flushed, pending
                while pending and pending[0][0] + EPB <= upto_excl:
                    c0, ps = pending.pop(0)
                    if out_sb is None:
                        out_sb = out_pool.tile([S, FLUSH * 2 * D], F32)
                        out_view = out_sb[:].rearrange(
                            "p (c two d) -> p c two d", two=2, d=D
                        )
                    k0 = c0 - flushed
                    nc.scalar.copy(out_view[:, k0 : k0 + EPB, 1, :], ps[:])
                    nc.vector.tensor_tensor(
                        out_view[:, k0 : k0 + EPB, 0, :],
                        ps[:],
                        recip_sb[:, c0 : c0 + EPB].to_broadcast((S, EPB, D)),
                        mybir.AluOpType.mult,
                    )
                    cl = c0 + EPB
                    if cl == C or cl % FLUSH == 0:
                        q0, q1 = flushed * 2 * D, cl * 2 * D
                        out_eng = nc.sync if cl == C else nc.gpsimd
                        out_eng.dma_start(
                            out_d[:, q0:q1], out_sb[:, 0 : q1 - q0]
                        )
                        flushed = cl
                        out_sb = None

            cbase = 0
            ps = None
            for nb in range(NB):
                k = BSIZES[nb]
                xt = x_pool.tile([P, DMA_BATCH * T * D], BF16)
                dma_eng = nc.scalar if nb % 2 == 0 else nc.sync
                # per-batch rel tile on the same ring just before its x
                # (tiny; separate tiles keep the one-hot dep precise)
                rel_t = rel_pool.tile([P, DMA_BATCH * T], I8)
                dma_eng.dma_start(
                    rel_t[:, : k * T], rel_d[:, cbase * T : (cbase + k) * T]
                )
                xoff = cbase * P * T * D
                dma_eng.dma_start(
                    xt[:, : k * T * D],
                    x_d[xoff : xoff + P * k * T * D].rearrange(
                        "(p c) -> p c", p=P
                    ),
                )
                oh = oh_pool.tile([P, DMA_BATCH * T * S], BF16)
                nc.vector.tensor_tensor(
                    oh[:, : k * T * S],
                    rel_t[:, : k * T].to_broadcast((P, k * T, S)),
                    iota_sb[:].unsqueeze(1).broadcast_to((P, k * T, S)),
                    mybir.AluOpType.is_equal,
                )
                for b in range(k):
                    c = cbase + b
                    if c % EPB == 0:
                        ps = ps_pool.tile([S, EPB * D], F32)
                        pending.append((c, ps))
                    w = (c % EPB) * D
                    for t in range(T):
                        nc.tensor.matmul(
                            ps[:, w : w + D],
                            oh[:, (b * T + t) * S : (b * T + t + 1) * S],
                            xt[:, (b * T + t) * D : (b * T + t + 1) * D],
                            start=(t == 0),
                            stop=(t == T - 1),
                        )
                # epilogues for banks finished before this batch
                emit_epilogue(cbase)
                cbase += k
            emit_epilogue(C)

    if split_syncs:
        _split_syncs(nc)
    return nc


def _greedy_plan(counts):
    """Pack consecutive segments into chunks with <=S segments and <=T*128
    rows, scanning candidate capacities T to minimize total padded rows.
    Returns (T, bases, nsegs) arrays (unpadded chunk list)."""
    g_total = len(counts)
    t_min = max(1, int(-(-int(counts.max()) // P)))
    # aim for ~S-3 segments per chunk so the S-slot cap rarely binds
    t_avg = max(t_min, -(-int(counts.sum()) * (S - 3) // (g_total * P)))
    best = None
    for T in range(max(t_min, t_avg - 3), max(t_min, t_avg) + 4):
        cap = T * P
        bases, nsegs = [], []
        g = 0
        r = 0
        n = 0
        while g + n < g_total:
            cnt = counts[g + n]
            if n < S and r + cnt <= cap:
                r += cnt
                n += 1
            else:
                assert n > 0, "single segment exceeds chunk capacity"
                bases.append(g)
                nsegs.append(n)
                g += n
                r = 0
                n = 0
        if n > 0:
            bases.append(g)
            nsegs.append(n)
        ct = len(bases)
        c_per = -(-ct // (N_CORES * FLUSH_ALIGN)) * FLUSH_ALIGN
        total = c_per * N_CORES * cap
        if best is None or total < best[0]:
            best = (total, T, np.array(bases), np.array(nsegs))
    _, T, bases, nsegs = best
    return T, bases, nsegs


def _plan_and_pack(x, seg):
    """Host-side: greedy chunk plan + packed/padded device arrays."""
    x = np.ascontiguousarray(x, dtype=np.float32)
    seg = np.asarray(seg).astype(np.int64)

    counts = np.bincount(seg, minlength=G).astype(np.int64)
    seg_row_start = np.zeros(G + 1, dtype=np.int64)
    np.cumsum(counts, out=seg_row_start[1:])
    recip = (1.0 / np.maximum(counts, 1.0)).astype(np.float32)

    T, bases, nsegs = _greedy_plan(counts)
    C = -(-len(bases) // (N_CORES * FLUSH_ALIGN)) * FLUSH_ALIGN  # chunks per core
    ct_pad = C * N_CORES
    pad = ct_pad - len(bases)
    # empty padding chunks (0 segments, 0 rows)
    bases_p = np.concatenate([bases, np.zeros(pad, dtype=np.int64)])
    nsegs_p = np.concatenate([nsegs, np.zeros(pad, dtype=np.int64)])
    row_start = seg_row_start[bases_p]
    n_rows = seg_row_start[bases_p + nsegs_p] - row_start

    # row index for [chunk, partition, tile]: row = start_c + t*128 + p
    ridx = (
        row_start[:, None, None]
        + np.arange(P, dtype=np.int64)[None, :, None]
        + (np.arange(T, dtype=np.int64) * P)[None, None, :]
    )
    valid = ridx < (row_start + n_rows)[:, None, None]
    ridx_c = np.where(valid, ridx, 0)

    # flat chunk-major per-partition layout: [P, C*T*D] per core; any batch
    # of consecutive chunks is a contiguous per-partition line
    ridx_b = ridx_c.transpose(1, 0, 2)          # [P, ct, T]
    valid_b = valid.transpose(1, 0, 2)
    xg = x[ridx_b.reshape(-1)].reshape(P, ct_pad, T, D)
    xg[~valid_b] = 0.0
    xbuf = xg.astype(NP_BF16)                   # [P, ct, T, D]
    del xg

    rel = seg[ridx_c] - bases_p[:, None, None]
    relbuf = np.where(valid, rel, -1).astype(np.int8)

    iota_np = np.tile(np.arange(S, dtype=np.int8), (P, 1))

    # per-slot reciprocal: slot s of chunk c -> segment bases[c]+s (1.0 pad)
    gidx = bases_p[:, None] + np.arange(S, dtype=np.int64)[None, :]
    slot_valid = np.arange(S)[None, :] < nsegs_p[:, None]
    recip_slots = np.where(
        slot_valid, recip[np.clip(gidx, 0, G - 1)], np.float32(1.0)
    ).astype(np.float32)

    in_maps = []
    for core in range(N_CORES):
        c0, c1 = core * C, (core + 1) * C
        rel_core = relbuf[c0:c1].transpose(1, 0, 2).reshape(P, C * T)
        xcore = xbuf[:, c0:c1]                   # [P, C, T, D]
        blocks = []
        cb = 0
        for k in _bsizes(C):
            blocks.append(
                np.ascontiguousarray(xcore[:, cb : cb + k]).reshape(-1)
            )
            cb += k
        in_maps.append(
            {
                "x": np.concatenate(blocks),
                "rel": np.ascontiguousarray(rel_core),
                "iota": iota_np,
                "recip": np.ascontiguousarray(recip_slots[c0:c1].T),
            }
        )
    plan = dict(T=T, C=C, gidx=gidx, slot_valid=slot_valid)
    return plan, in_maps


def _assemble(results, plan):
    """[core]["out"] of shape [S, C*2*D] -> [G, 2*D] via the slot->segment map."""
    C = plan["C"]
    vs = [results[core]["out"].reshape(S, C, 2, D) for core in range(N_CORES)]
    mean = np.concatenate([v[:, :, 0, :].transpose(1, 0, 2) for v in vs])  # [ct,S,D]
    ssum = np.concatenate([v[:, :, 1, :].transpose(1, 0, 2) for v in vs])
    out = np.empty((G, 2 * D), np.float32)
    m = plan["slot_valid"]
    out[plan["gidx"][m], :D] = mean[m]
    out[plan["gidx"][m], D:] = ssum[m]
    return out


def _run_impl(nbr_fea, segment_ids, num_segments, trace=False, trace_kwargs=None):
    assert int(num_segments) == G, f"expected {G} segments, got {num_segments}"
    assert nbr_fea.shape == (N_TOTAL, D), nbr_fea.shape

    plan, in_maps = _plan_and_pack(nbr_fea, segment_ids)
    nc = _build_bass(plan["T"], plan["C"])
    kw = {}
    if trace:
        kw = dict(trace=True, **(trace_kwargs or {}))
    res = bass_utils.run_bass_kernel_spmd(
        nc, in_maps, core_ids=list(range(N_CORES)), **kw
    )
    return _assemble(res.results, plan), res


def kernel(nbr_fea, segment_ids, num_segments):
    out, _ = _run_impl(np.asarray(nbr_fea), np.asarray(segment_ids), num_segments)
    return out
